# revision 29
# baseline (speedup 1.0000x reference)
"""Trainium2 Bass kernel for nn_CCepLTVFilter (fp16 PE pipeline, v4).

Per core (frequency-sharded, f-slice of 128 across 8 cores):
  1. conv1d(x, W) + b: 3 taps as shifted views of a zero-padded x panel
     (im2col done by APs, not by duplicating bytes)      (PE fp16, K=80 x3)
  2. Yq/Yi = DFT of padded ccep  -> [f, bt]              (PE fp16; CF has
     ln10/40 and 1/qnorm folded so mag = exp(4*Yq))
  3. mag = ((1+poly3(Yq))^2)^2 via 2 ACT Squares; sin/cos(Yi) via DVE
     range-wrap + ACT Sin (single trig table set)
  4. Zr/Zi = 1025-point DFT of z frames; the 50%-overlap frames come from
     PE transposes of the *unduplicated* z panel (row-shifted views give
     the hop offset)                                     (PE fp16)
  5. qr/qi = (cos + i sin)(Zr + i Zi); P = mag * q       (DVE)
  6. zf_b[t,w] = Pr_b.T @ CO + Pi_b.T @ SO (win/M folded) (PE fp16)
Output: per-core zf [B,T,WIN] fp32 partial spectra summed on host, then the
overlap-add (linear) runs on host in fp32.

Input DMA is the wall here (sustained DRAM-read rate per queue is low), so
inputs are minimized: unduplicated z (128K), compact x+w (150K), fp16
trig tables (625K). Three queues are ordered by first use. A junk-matmul
warmup keeps the PE clock ramping while DMAs land; a dummy Sin pulls the
single ACT table load off the critical path.
"""

import numpy as np

import concourse.bass as bass
import concourse.bacc as bacc
import concourse.mybir as mybir
import concourse.tile as tile
from concourse.bass_utils import run_bass_kernel_spmd

# ---------------- problem dims (hardcoded) ----------------
B, T, D = 2, 128, 80
CCEP = 222
FFT = 1024
HOP = 256
WIN = 2 * HOP            # 512
PAD = (FFT - CCEP) // 2  # 401
M = FFT + 1              # 1025-point transforms
BT = B * T               # 256
NCORES = 8
FS = FFT // NCORES       # 128 frequencies per core
OC = CCEP // 2           # 111 (o-chunk)
LAM = float(np.log(10.0) / 10.0)

F32 = mybir.dt.float32
F16 = mybir.dt.float16
PI = float(np.pi)

NWARM = 8               # junk matmuls to keep PE busy while DMAs land

TRACE = False            # set by test harness for profiling
LAST_RESULT = None       # BassKernelResults of last run (for test harness)


# ---------------- host-side constants (input independent) ----------------
def _make_constants():
    o = np.arange(CCEP, dtype=np.float64)[:, None]
    f = np.arange(FFT, dtype=np.float64)[None, :]
    qn_idx = np.arange(1, CCEP // 2 + 1, dtype=np.float64)
    qnorm = np.concatenate([qn_idx[::-1], qn_idx])
    ang = 2.0 * np.pi * f * (o + PAD) / FFT
    CF = np.cos(ang) * (LAM / 4.0) / qnorm[:, None]      # [222,1024]
    SF = -np.sin(ang) / qnorm[:, None]

    u = np.arange(WIN, dtype=np.float64)[:, None]
    phi = 2.0 * np.pi * f * (u + FFT // 2) / M
    ZC = np.cos(phi)                                     # [512,1024]
    ZS = np.sin(phi)

    w = np.arange(WIN, dtype=np.float64)[None, :]
    th = 2.0 * np.pi * np.arange(FFT, dtype=np.float64)[:, None] * w / M
    win = 0.5 * (1.0 - np.cos(2.0 * np.pi * np.arange(WIN) / WIN))
    CO = np.cos(th) * win[None, :] / M                   # [1024,512]
    SO = np.sin(th) * win[None, :] / M

    consts = []
    for c in range(NCORES):
        sl = slice(c * FS, (c + 1) * FS)
        cfp = CF[:, sl].reshape(2, OC, FS).transpose(1, 0, 2).reshape(OC, 2 * FS)
        sfp = SF[:, sl].reshape(2, OC, FS).transpose(1, 0, 2).reshape(OC, 2 * FS)
        # 2 trailing columns reserved for the conv bias (filled per call)
        cpack1 = np.concatenate(
            [cfp, sfp, np.zeros((OC, 2))], axis=1).astype(np.float16)
        zcp = ZC[:, sl].reshape(4, 128, FS).transpose(1, 0, 2).reshape(128, 4 * FS)
        zsp = ZS[:, sl].reshape(4, 128, FS).transpose(1, 0, 2).reshape(128, 4 * FS)
        consts.append(dict(cpack1=cpack1,
                           cpack2a=zcp.astype(np.float16),
                           cpack2b=zsp.astype(np.float16),
                           cpack3a=CO[sl, :].astype(np.float16),
                           cpack3b=SO[sl, :].astype(np.float16)))
    return consts


_CONSTS = _make_constants()
_NC = None

XW = 260                 # padded x panel width: z | b0(128) | z z | b1(128) | z


# ---------------- device program ----------------
def _build_nc():
    nc = bacc.Bacc()
    # spack rows: [x panel as (2,130) | w2 taps (3*222)] fp16, [80, 926]
    sp_e = nc.dram_tensor("spack", [D, 2, XW // 2 + 333], F16,
                          kind="ExternalInput")
    zn_e = nc.dram_tensor("znat", [T, B * HOP], F16, kind="ExternalInput")
    # cpack1 = [cf | sf | bias(2 cols)] fp16, [111, 514]
    c1_e = nc.dram_tensor("cpack1", [OC, 4 * FS + 2], F16, kind="ExternalInput")
    c2a_e = nc.dram_tensor("cpack2a", [128, 4 * FS], F16, kind="ExternalInput")
    c2b_e = nc.dram_tensor("cpack2b", [128, 4 * FS], F16, kind="ExternalInput")
    c3a_e = nc.dram_tensor("cpack3a", [128, 4 * FS], F16, kind="ExternalInput")
    c3b_e = nc.dram_tensor("cpack3b", [128, 4 * FS], F16, kind="ExternalInput")
    zf_e = nc.dram_tensor("zfo", [B, T, WIN], F32, kind="ExternalOutput")

    AOP = mybir.AluOpType
    SIN = mybir.ActivationFunctionType.Sin
    SQ = mybir.ActivationFunctionType.Square
    IDF = mybir.ActivationFunctionType.Identity

    with tile.TileContext(nc) as tc:
        with tc.tile_pool(name="sb", bufs=1) as sb, \
             tc.tile_pool(name="ps", bufs=2, space="PSUM") as ps:

            # ---- input DMAs: 3 queues, load-balanced + ordered by use ----
            # spack 3D [80, 2, 463]: per block: x-half (130) | w2-half (333)
            spack = sb.tile([D, 2, XW // 2 + 333], F16, tag="spack",
                            name="spack")
            nc.scalar.dma_start(out=spack[:], in_=sp_e[:, :])
            cp1 = sb.tile([OC, 4 * FS + 2], F16, tag="cp1", name="cp1")
            nc.scalar.dma_start(out=cp1[:], in_=c1_e[:, :])
            znat = sb.tile([T, B * HOP], F16, tag="znat", name="znat")
            nc.sync.dma_start(out=znat[:], in_=zn_e[:, :])
            zc = sb.tile([128, 4 * FS], F16, tag="zc", name="zc")
            nc.sync.dma_start(out=zc[:], in_=c2a_e[:, :])
            zs = sb.tile([128, 4 * FS], F16, tag="zs", name="zs")
            nc.gpsimd.dma_start(out=zs[:], in_=c2b_e[:, :])
            co = sb.tile([128, 4 * FS], F16, tag="co", name="co")
            nc.gpsimd.dma_start(out=co[:], in_=c3a_e[:, :])
            so = sb.tile([128, 4 * FS], F16, tag="so", name="so")
            nc.gpsimd.dma_start(out=so[:], in_=c3b_e[:, :])

            cf = cp1[:, 0:2 * FS]
            sf = cp1[:, 2 * FS:4 * FS]

            # ---- on-chip identity (fp16, exact) for PE transposes ----
            idt = sb.tile([128, 128], F16, tag="idt", name="idt")
            nc.gpsimd.memset(idt[:, :], 1.0)
            nc.gpsimd.affine_select(
                out=idt[:, :], in_=idt[:, :],
                compare_op=AOP.is_equal, fill=0.0,
                base=0, pattern=[[-1, 128]], channel_multiplier=1)

            # junk tile for PE warmup + dummy ACT-Sin input (forces the
            # single trig table load early, overlapped with input DMA)
            junk = sb.tile([128, 128], F16, tag="junk", name="junk")
            nc.vector.memset(junk[:, :], 0.25)
            adum = sb.tile([128, 2], F32, tag="adum", name="adum")
            nc.vector.memset(adum[:, :], 0.0)
            asin = sb.tile([128, 2], F32, tag="asin", name="asin")
            nc.scalar.activation(asin[:, :], adum[:, :], SIN)

            # frames tile; t=0 columns of the low-half chunks stay zero
            fr = sb.tile([128, 4 * BT], F16, tag="fr", name="fr")
            for mc in range(2):
                for bb in range(B):
                    nc.vector.memset(
                        fr[:, mc * BT + bb * T: mc * BT + bb * T + 1], 0.0)

            # ---- PE warmup: junk matmuls while DMAs land ----
            junkp = ps.tile([128, 64], F32, tag="tpB", bufs=2, name="junkp")
            for _ in range(NWARM):
                nc.tensor.matmul(junkp[:, :], junk[:, :], junk[:, 0:64],
                                 start=True, stop=True)

            # ---- conv: ccep[o, bt] = sum_k Wk.T @ x[t+k-1]; bias via the
            # ccep copy (ACT Identity with per-partition bias) ----
            ccep = []
            XH = XW // 2  # 130
            for c in range(2):
                pc = ps.tile([OC, BT], F32, tag="tpB", bufs=2, name=f"conv{c}")
                for k in range(3):
                    idx = k * CCEP + c * OC
                    blk, off = idx // 333, idx % 333
                    lhs = spack[:, blk, XH + off: XH + off + OC]
                    rhs = spack[:, :, k:k + T]
                    nc.tensor.matmul(pc[:, :], lhs, rhs,
                                     start=(k == 0), stop=(k == 2))
                cs = sb.tile([OC, BT], F16, tag=f"ccep{c}", name=f"ccep{c}")
                nc.scalar.activation(cs[:, :], pc[:, :], IDF,
                                     bias=cp1[0:OC, 4 * FS + c: 4 * FS + c + 1])
                ccep.append(cs)

            # ---- frames via PE transposes of znat (fp16) ----
            # fr[u, mc*BT + b*T + t]:
            #   mc>=2: = znat[t, b*HOP + (mc-2)*128 + u']  (direct transpose)
            #   mc<2 : = znat[t-1, b*HOP + mc*128 + u'] (row-shifted; t=0 -> 0)
            for mc in range(4):
                for bb in range(B):
                    if mc >= 2:
                        src = znat[:, bb * HOP + (mc - 2) * 128:
                                   bb * HOP + (mc - 1) * 128]
                        tp = ps.tile([128, T], F16, tag="tpA", bufs=2,
                                     name=f"ftp{mc}{bb}")
                        nc.tensor.transpose(tp[:, :], src, idt[:, :])
                        dstv = fr[:, mc * BT + bb * T: mc * BT + (bb + 1) * T]
                        cw = T
                    else:
                        src = znat[0:T - 1, bb * HOP + mc * 128:
                                   bb * HOP + (mc + 1) * 128]
                        tp = ps.tile([128, T], F16, tag="tpA", bufs=2,
                                     name=f"ftp{mc}{bb}")
                        nc.tensor.transpose(tp[:, 0:T - 1], src,
                                            idt[0:T - 1, 0:T - 1])
                        dstv = fr[:, mc * BT + bb * T + 1:
                                  mc * BT + (bb + 1) * T]
                        cw = T - 1
                    nc.vector.tensor_copy(dstv, tp[:, 0:cw])

            # ---- Yq/Yi [f_local, bt] (LAM/4 folded into CF) ----
            yr = ps.tile([FS, BT], F32, tag="tpC", bufs=4, name="yr")
            yi = ps.tile([FS, BT], F32, tag="tpC", bufs=4, name="yi")
            for c in range(2):
                nc.tensor.matmul(yr[:, :], cf[:, c * FS:(c + 1) * FS],
                                 ccep[c][:, :], start=(c == 0), stop=(c == 1))
            for c in range(2):
                nc.tensor.matmul(yi[:, :], sf[:, c * FS:(c + 1) * FS],
                                 ccep[c][:, :], start=(c == 0), stop=(c == 1))

            # ---- Zr/Zi [f_local, bt] ----
            zr = ps.tile([FS, BT], F32, tag="tpC", bufs=4, name="zr")
            zi = ps.tile([FS, BT], F32, tag="tpC", bufs=4, name="zi")
            for mc in range(4):
                nc.tensor.matmul(zr[:, :], zc[:, mc * FS:(mc + 1) * FS],
                                 fr[:, mc * BT:(mc + 1) * BT],
                                 start=(mc == 0), stop=(mc == 3))
            for mc in range(4):
                nc.tensor.matmul(zi[:, :], zs[:, mc * FS:(mc + 1) * FS],
                                 fr[:, mc * BT:(mc + 1) * BT],
                                 start=(mc == 0), stop=(mc == 3))

            def wtile(name):
                return sb.tile([FS, BT], F32, tag=name, name=name)

            # ---- DVE chain: poly exp interleaved with the Yi wraps ----
            eu = wtile("eu")
            ev = wtile("ev")
            nc.vector.tensor_scalar_mul(eu[:, :], yr[:, :], 1.0 / 6.0)
            yiw = wtile("yiw")
            nc.vector.add_range_wrap(yiw[:, :], yi[:, :], 0.0, PI, 2.0 * PI)
            nc.vector.scalar_tensor_tensor(ev[:, :], eu[:, :], 0.5,
                                           yr[:, :], AOP.add, AOP.mult)
            yic = wtile("yic")
            nc.vector.add_range_wrap(yic[:, :], yi[:, :], PI / 2.0, PI, 2.0 * PI)
            nc.vector.scalar_tensor_tensor(eu[:, :], ev[:, :], 1.0,
                                           yr[:, :], AOP.add, AOP.mult)

            sinv = wtile("sinv")
            nc.scalar.activation(sinv[:, :], yiw[:, :], SIN)
            cosv = wtile("cosv")
            nc.scalar.activation(cosv[:, :], yic[:, :], SIN)
            sq1 = wtile("sq1")
            nc.scalar.activation(sq1[:, :], eu[:, :], SQ, bias=1.0)
            mag = wtile("mag")
            nc.scalar.activation(mag[:, :], sq1[:, :], SQ)

            # ---- q = (cos + i sin)(Zr + i Zi); P = mag*q (fp16 out) ----
            m1 = wtile("m1")
            nc.vector.tensor_tensor(m1[:, :], cosv[:, :], zr[:, :], AOP.mult)
            m2 = wtile("m2")
            nc.vector.tensor_tensor(m2[:, :], sinv[:, :], zi[:, :], AOP.mult)
            qr = wtile("qr")
            nc.vector.tensor_tensor(qr[:, :], m1[:, :], m2[:, :], AOP.subtract)
            Pr = sb.tile([FS, BT], F16, tag="Pr", name="Pr")
            nc.vector.tensor_tensor(Pr[:, :], mag[:, :], qr[:, :], AOP.mult)
            m3 = wtile("m3")
            nc.vector.tensor_tensor(m3[:, :], cosv[:, :], zi[:, :], AOP.mult)
            m4 = wtile("m4")
            nc.vector.tensor_tensor(m4[:, :], sinv[:, :], zr[:, :], AOP.mult)
            qi = wtile("qi")
            nc.vector.tensor_tensor(qi[:, :], m3[:, :], m4[:, :], AOP.add)
            Pi = sb.tile([FS, BT], F16, tag="Pi", name="Pi")
            nc.vector.tensor_tensor(Pi[:, :], mag[:, :], qi[:, :], AOP.mult)

            # ---- step6: zf_b[t,w] = Pr_b.T @ CO + Pi_b.T @ SO ----
            zfbs = []
            for bb in range(B):
                zfb = ps.tile([T, WIN], F32, tag="tpC", bufs=4, name=f"zfb{bb}")
                nc.tensor.matmul(zfb[:, :], Pr[:, bb * T:(bb + 1) * T], co[:, :],
                                 start=True, stop=False)
                zfbs.append(zfb)
            for bb in range(B):
                nc.tensor.matmul(zfbs[bb][:, :], Pi[:, bb * T:(bb + 1) * T], so[:, :],
                                 start=False, stop=True)
            for bb in range(B):
                zfo = sb.tile([T, WIN], F32, tag=f"zfo{bb}", name=f"zfo{bb}")
                # split the PSUM->SBUF copy across ACT and DVE
                nc.scalar.copy(zfo[:, 0:HOP], zfbs[bb][:, 0:HOP])
                nc.vector.tensor_copy(zfo[:, HOP:WIN], zfbs[bb][:, HOP:WIN])
                eng = nc.sync if bb == 0 else nc.scalar
                dst = bass.AP(zf_e[:, :, :].tensor, bb * T * WIN,
                              [[WIN, T], [1, WIN]])
                eng.dma_start(out=dst, in_=zfo[:, :])

    return nc


def _get_nc():
    global _NC
    if _NC is None:
        _NC = _build_nc()
        _NC.finalize()
    return _NC


# ---------------- host orchestration ----------------
def kernel(x, z, W, b):
    global LAST_RESULT
    x = np.ascontiguousarray(np.asarray(x, dtype=np.float32))
    z = np.ascontiguousarray(np.asarray(z, dtype=np.float32))
    W = np.ascontiguousarray(np.asarray(W, dtype=np.float32))
    b = np.ascontiguousarray(np.asarray(b, dtype=np.float32))

    # x panel [80, 260]: cols 0 zero | b0 t0..127 | zero zero | b1 | zero
    xT = x.reshape(BT, D).T                                       # [80, 256]
    xpan = np.zeros((D, XW), np.float32)
    xpan[:, 1:1 + T] = xT[:, 0:T]
    xpan[:, 3 + T:3 + 2 * T] = xT[:, T:2 * T]
    # taps: ccep[bt] = sum_k W[:,:,k].T @ x[t+k-1]
    w2 = np.concatenate([W[:, :, 0], W[:, :, 1], W[:, :, 2]],
                        axis=0).reshape(3 * CCEP, D).T            # [80, 666]
    spack = np.zeros((D, 2, XW // 2 + 333), np.float32)
    spack[:, :, 0:XW // 2] = xpan.reshape(D, 2, XW // 2)
    spack[:, :, XW // 2:] = w2.reshape(D, 2, 333)
    spack = spack.astype(np.float16)

    # unduplicated z panel [t, (b, j)]: znat[t, b*HOP+j] = z[b, t*HOP+j]
    znat = np.ascontiguousarray(
        z[:, 0, :].reshape(B, T, HOP).transpose(1, 0, 2).reshape(T, B * HOP)
    ).astype(np.float16)

    shared = {"spack": spack, "znat": znat}
    in_maps = []
    for c in range(NCORES):
        cp = dict(_CONSTS[c])
        c1 = cp["cpack1"].copy()
        c1[:, 4 * FS] = b[0:OC].astype(np.float16)
        c1[:, 4 * FS + 1] = b[OC:CCEP].astype(np.float16)
        cp["cpack1"] = c1
        in_maps.append({**shared, **cp})

    nc = _get_nc()
    res = run_bass_kernel_spmd(nc, in_maps, list(range(NCORES)), trace=TRACE)
    LAST_RESULT = res
    zf = np.zeros((B, T, WIN), dtype=np.float32)
    for r in res.results:
        zf += np.asarray(r["zfo"], dtype=np.float32)
    # overlap-add on host (linear, fp32): o[t] = l[t] + r[t-1 mod T]
    l, r = zf[:, :, :HOP], zf[:, :, HOP:]
    out = l + np.roll(r, 1, axis=1)
    return out.reshape(B, 1, T * HOP)
